# revision 12
# baseline (speedup 1.0000x reference)
"""Trainium2 Bass kernel for nn_GATPolicy (3x bipartite GATConv + cartesian MLP).

8 NeuronCores, SPMD. Key restructurings (all exact up to fp rounding):
  * Products (8192) row-sharded 1024/core; users/personas replicated;
    head sharded 32 users/core.
  * Graph edges -> dense log-count masks on the host:
    LC[dst,src] = ln(multiplicity), -200 if absent.  Segment softmax becomes
    dense masked softmax (duplicate edges share identical logits so they fold
    into ln(count); exp(x-200) underflows to exact +0).  Aggregation becomes
    out = softmax(lrelu(as+ad)+LC) @ xs - dense matmuls.
  * xd is only needed through per-node scores: ad = x_dst @ (Wd@att_d),
    folded on the host.  edge_attr == 0 so the We path vanishes.
  * Activations feature-major (T[feat, node]); weights [K,M] used directly as
    the stationary matmul operand; biases fused into PSUM-evict activations.
  * v2c/p2p contract over the sharded product dim: one AllReduce per net of
    numerator[1536,Nd]+denominator[2,Nd]; softmax normalization after.
  * Head fc0 factorized over the cartesian product; eval-BN folded into
    weights host-side.
  * Matmuls in float32r (tf32-class, ~1.6e-4), fp32 accumulate.
"""
import numpy as np

import concourse.bass as bass
import concourse.tile as tile
from concourse import bacc, mybir
from concourse.alu_op_type import AluOpType
from concourse.bass_utils import run_bass_kernel_spmd

dt = mybir.dt
F32 = dt.float32
F32R = dt.float32r
Act = mybir.ActivationFunctionType

D = 768
H = 2
D2 = 1536
NU = 256
NP = 8192
NK = 128
F_IN = 3551
FP = 3584            # F_IN zero-padded to 28*128
NCORE = 8
PC = NP // NCORE     # 1024 products/core
UC = NU // NCORE     # 32 users/core
NPAIR = UC * NK      # 4096 pairs/core
DC = D // 128        # 6
D2C = D2 // 128      # 12
KC = FP // 128       # 28
PCC = PC // 128      # 8
UCC = NU // 128      # 2
MASK = np.float32(-200.0)


def _nt(N, nt=512):
    out, n0 = [], 0
    while n0 < N:
        nn = min(nt, N - n0)
        out.append((n0, nn))
        n0 += nn
    return out


def _build(debug=False):
    nc = bacc.Bacc("TRN2", target_bir_lowering=False, debug=False,
                   num_devices=NCORE)

    def inp(name, shape):
        return nc.dram_tensor(name, list(shape), F32, kind="ExternalInput").ap()

    def outp(name, shape):
        return nc.dram_tensor(name, list(shape), F32, kind="ExternalOutput").ap()

    d = {}
    d["ones"] = inp("c_ones", [128, 128])
    d["ufT"] = inp("ufT", [FP, NU])
    d["kfT"] = inp("kfT", [FP, NK])
    for nm in ("W_user", "W_prod", "W_pers"):
        d[nm] = inp(nm, [FP, D])
    for nm in ("b_user", "b_prod", "b_pers"):
        d[nm] = inp(nm, [D, 1])
    for g in range(3):
        d[f"g{g}_Ws"] = inp(f"g{g}_Ws", [D, D2])
        d[f"g{g}_Wsatt"] = inp(f"g{g}_Wsatt", [D, H])
        d[f"g{g}_Wdatt"] = inp(f"g{g}_Wdatt", [D, H])
        d[f"g{g}_gbias"] = inp(f"g{g}_gbias", [D2, 1])
        d[f"g{g}_ltW"] = inp(f"g{g}_ltW", [D2, D])
        d[f"g{g}_ltb"] = inp(f"g{g}_ltb", [D, 1])
        d[f"g{g}_ffW"] = inp(f"g{g}_ffW", [D, D])
        d[f"g{g}_ffb"] = inp(f"g{g}_ffb", [D, 1])
        d[f"g{g}_pcW"] = inp(f"g{g}_pcW", [D2, D])
        d[f"g{g}_pcb"] = inp(f"g{g}_pcb", [D, 1])
    d["fc0a"] = inp("fc0a", [D, D])
    d["b0"] = inp("b0", [D, 1])
    d["fc0b"] = inp("fc0b", [D, D])
    d["fc1W"] = inp("fc1W", [D, 384])
    d["fc1b"] = inp("fc1b", [384, 1])
    d["fc2W"] = inp("fc2W", [384, 128])
    d["fc2b"] = inp("fc2b", [128, 1])
    d["fc3W"] = inp("fc3W", [128, 1])
    d["fc3b"] = inp("fc3b", [1, 1])
    d["pfT"] = inp("pfT", [FP, PC])
    d["LC1T"] = inp("LC1T", [NU, PC])
    d["LC2"] = inp("LC2", [PC, NU])
    d["LC3"] = inp("LC3", [PC, NK])
    d["Ssel"] = inp("Ssel", [NU, UC])
    d["y"] = outp("y", [1, NPAIR])
    d["u1T"] = outp("u1T", [D, NU])
    d["pr1T"] = outp("pr1T", [D, PC])

    with tile.TileContext(nc) as tc:
        _emit(nc, tc, d, debug)
    nc.compile()
    return nc


def _emit(nc, tc, d, debug):
    from contextlib import ExitStack
    ctx = ExitStack()
    glob = ctx.enter_context(tc.tile_pool(name="glob", bufs=1))
    wk = ctx.enter_context(tc.tile_pool(name="wk", bufs=6))
    ps = ctx.enter_context(tc.tile_pool(name="ps", bufs=1, space="PSUM"))
    drm = ctx.enter_context(tc.tile_pool(name="drm", bufs=1, space="DRAM"))

    def pmm(shape):
        return ps.tile(shape, F32, name="mmp", tag="mm", bufs=3)

    def pbc(shape):
        return ps.tile(shape, F32, name="bcp", tag="bc", bufs=2)

    def pdn(shape):
        return ps.tile(shape, F32, name="dnp", tag="dn", bufs=1)

    def dump(name, tiles, N, parts=128):
        if not debug:
            return
        ap = nc.dram_tensor("dbg_" + name, [len(tiles) * parts, N], F32,
                            kind="ExternalOutput").ap()
        for i, t in enumerate(tiles):
            src = t[0:parts, 0:N]
            if src.dtype == F32R:
                src = src.bitcast(F32)
            nc.sync.dma_start(ap[i * parts:(i + 1) * parts, :], src)

    def stage_bias(dap, M, name):
        mc = -(-M // 128)
        t = glob.tile([128, mc, 1], F32, name="b_" + name)
        nc.sync.dma_start(t[:], dap.rearrange("(a p) o -> p a o", p=128))
        return t

    def stage_km(pool, dap, K, M, name, wdt=F32R, tag="", bufs=1):
        t = pool.tile([128, K // 128, M], wdt, name=name, tag=tag or name, bufs=bufs)
        src = dap.rearrange("(a p) m -> p a m", p=128)
        nc.sync.dma_start(t[:], src.bitcast(wdt) if wdt != F32 else src)
        return t

    def ft_gemm(Wd, K, M, rhs_at, N, evict, live=2):
        """out chunks [mi, n0:nn] = W[K,M].T @ X[K,N], W streamed from DRAM."""
        kc = K // 128
        tiles = _nt(N)
        for mi in range(M // 128):
            for gs in range(0, len(tiles), live):
                grp = tiles[gs:gs + live]
                pos = [pmm([128, nn]) for (_, nn) in grp]
                for ki in range(kc):
                    w = wk.tile([128, 128], F32R, name="wchunk")
                    nc.sync.dma_start(
                        w[:], Wd[ki * 128:(ki + 1) * 128,
                                 mi * 128:(mi + 1) * 128].bitcast(F32R))
                    for po, (n0, nn) in zip(pos, grp):
                        nc.tensor.matmul(po[:], w[:], rhs_at(ki, n0, nn),
                                         start=(ki == 0), stop=(ki == kc - 1))
                for po, (n0, nn) in zip(pos, grp):
                    evict(po, mi, n0, nn)

    def ev_bias(out_tiles, func, biasS, coff=0):
        def ev(po, mi, n0, nn):
            nc.scalar.activation(out_tiles[mi][:, coff + n0:coff + n0 + nn],
                                 po[:], func, bias=biasS[:, mi, :])
        return ev

    def scores_nm(pool, xT, wattS, nchunks, name):
        """as[node,h] node-major per 128-chunk: x @ (W@att)."""
        out = []
        for s in range(nchunks):
            po = pmm([128, H])
            for ki in range(DC):
                nc.tensor.matmul(po[:], xT[ki][:, s * 128:(s + 1) * 128],
                                 wattS[:, ki, :], start=(ki == 0), stop=(ki == DC - 1))
            t = pool.tile([128, H], F32, name=f"{name}{s}", tag=name, bufs=nchunks)
            nc.scalar.copy(t[:], po[:])
            out.append(t)
        return out

    def scores_fr(pool, xT, wattS, N, name):
        """ad[h][node] feature-rows: (W@att).T @ xT, one [1,N] tile per head."""
        ts = [pool.tile([1, N], F32, name=f"{name}{h}") for h in range(H)]
        for h in range(H):
            for (n0, nn) in _nt(N):
                pd = pdn([1, nn])
                for ki in range(DC):
                    nc.tensor.matmul(pd[:], wattS[:, ki, h:h + 1],
                                     xT[ki][:, n0:n0 + nn],
                                     start=(ki == 0), stop=(ki == DC - 1))
                nc.scalar.copy(ts[h][:, n0:n0 + nn], pd[:])
        return ts

    def bcast(rowt, n0, nn):
        """Broadcast rowt[0, n0:n0+nn] to 128 partitions (K=1 ones-matmul)."""
        pb = pbc([128, nn])
        for (m0, mm) in _nt(nn):
            nc.tensor.matmul(pb[:, m0:m0 + mm], ones_f[0:1, :],
                             rowt[0:1, n0 + m0:n0 + m0 + mm],
                             start=True, stop=True)
        return pb

    def bcast_sb(rowt, n0, nn, pool, tag):
        """bcast + evict to SBUF (for use as a DVE tensor_tensor operand)."""
        pb = bcast(rowt, n0, nn)
        t = pool.tile([128, nn], F32, name=tag, tag=tag, bufs=2)
        nc.scalar.copy(t[:], pb[:])
        return t

    # ---------------- constants
    ones_f = glob.tile([128, 128], F32, name="ones_f")
    nc.sync.dma_start(ones_f[:], d["ones"])
    eps1 = glob.tile([1, 1], F32, name="eps1")
    nc.vector.memset(eps1[:], 1e-16)
    ones_r = glob.tile([128, 128], F32R, name="ones_r")
    nc.sync.dma_start(ones_r[:], d["ones"].bitcast(F32R))
    bias_u = stage_bias(d["b_user"], D, "bu")
    bias_p = stage_bias(d["b_prod"], D, "bp")
    bias_k = stage_bias(d["b_pers"], D, "bk")
    gb, ltb, ffb, pcb, wsa, wda = [], [], [], [], [], []
    for g in range(3):
        gb.append(stage_bias(d[f"g{g}_gbias"], D2, f"g{g}gb"))
        ltb.append(stage_bias(d[f"g{g}_ltb"], D, f"g{g}lb"))
        ffb.append(stage_bias(d[f"g{g}_ffb"], D, f"g{g}fb"))
        pcb.append(stage_bias(d[f"g{g}_pcb"], D, f"g{g}pb"))
        wsa.append(stage_km(glob, d[f"g{g}_Wsatt"], D, H, f"g{g}wsa"))
        wda.append(stage_km(glob, d[f"g{g}_Wdatt"], D, H, f"g{g}wda"))
    b0S = stage_bias(d["b0"], D, "b0")
    fc1bS = stage_bias(d["fc1b"], 384, "fc1b")
    fc2bS = stage_bias(d["fc2b"], 128, "fc2b")
    fc3bS = glob.tile([1, 1], F32, name="fc3bS")
    nc.sync.dma_start(fc3bS[:], d["fc3b"])

    # persistent activations
    u0T = [glob.tile([128, NU], F32R, name=f"u0T{i}") for i in range(DC)]
    pe0T = [glob.tile([128, NK], F32R, name=f"pe0T{i}") for i in range(DC)]
    pr1T = [glob.tile([128, PC], F32R, name=f"pr1T{i}") for i in range(DC)]
    u1T = [glob.tile([128, NU], F32R, name=f"u1T{i}") for i in range(DC)]
    pe1T = [glob.tile([128, NK], F32R, name=f"pe1T{i}") for i in range(DC)]

    # ============ stage 1: embeddings ============
    with tc.tile_pool(name="pr0p", bufs=1) as pr0p:
        pr0T = [pr0p.tile([128, PC], F32R, name=f"pr0T{i}") for i in range(DC)]
        with tc.tile_pool(name="embA", bufs=1) as embA:
            ufS = embA.tile([128, KC, NU], F32R, name="ufS")
            nc.sync.dma_start(ufS[:], d["ufT"].bitcast(F32R)
                              .rearrange("(a p) n -> p a n", p=128))
            kfS = embA.tile([128, KC, NK], F32R, name="kfS")
            nc.sync.dma_start(kfS[:], d["kfT"].bitcast(F32R)
                              .rearrange("(a p) n -> p a n", p=128))
            ft_gemm(d["W_user"], FP, D,
                    lambda ki, n0, nn: ufS[:, ki, n0:n0 + nn], NU,
                    ev_bias(u0T, Act.Identity, bias_u))
            ft_gemm(d["W_pers"], FP, D,
                    lambda ki, n0, nn: kfS[:, ki, n0:n0 + nn], NK,
                    ev_bias(pe0T, Act.Identity, bias_k))
        with tc.tile_pool(name="embB", bufs=1) as embB:
            for half in range(2):
                pfS = embB.tile([128, KC, PC // 2], F32R, name="pfS",
                                tag="pfS", bufs=1)
                nc.sync.dma_start(
                    pfS[:], d["pfT"].bitcast(F32R)
                    .rearrange("(a p) n -> p a n", p=128)
                    [:, :, half * 512:(half + 1) * 512])
                ft_gemm(d["W_prod"], FP, D,
                        lambda ki, n0, nn: pfS[:, ki, n0:n0 + nn], PC // 2,
                        ev_bias(pr0T, Act.Identity, bias_p, coff=half * 512))
        dump("u0T", u0T, NU)
        dump("pe0T", pe0T, NK)
        dump("pr0T", pr0T, PC)

        # ============ stage 2: c2v (src=user, dst=product shard; local softmax)
        with tc.tile_pool(name="c2vT", bufs=1) as c2vT:
            gatT = [c2vT.tile([128, PC], F32R, name=f"c2vg{i}", tag="c2vg",
                              bufs=D2C) for i in range(D2C)]
            with tc.tile_pool(name="c2vA", bufs=1) as cA:
                asu = scores_nm(cA, u0T, wsa[0], UCC, "c2v_as")
                adrow = scores_fr(cA, pr0T, wda[0], PC, "c2v_ad")
                # xs node-major [256 x 1536] (weights streamed once)
                xsu = [cA.tile([128, D2], F32R, name=f"c2vxs{s}") for s in range(UCC)]
                for (n0, nn) in _nt(D2):
                    pos = [pmm([128, nn]) for _ in range(UCC)]
                    for ki in range(DC):
                        w = wk.tile([128, 512], F32R, name="wchunk2", tag="wchunk2",
                                    bufs=4)
                        nc.sync.dma_start(
                            w[:, 0:nn],
                            d["g0_Ws"][ki * 128:(ki + 1) * 128, n0:n0 + nn].bitcast(F32R))
                        for s in range(UCC):
                            nc.tensor.matmul(pos[s][:], u0T[ki][:, s * 128:(s + 1) * 128],
                                             w[:, 0:nn], start=(ki == 0),
                                             stop=(ki == DC - 1))
                    for s in range(UCC):
                        nc.scalar.copy(xsu[s][:, n0:n0 + nn], pos[s][:])
                LC1 = [cA.tile([128, PC], F32, name=f"c2vlc{s}") for s in range(UCC)]
                for s in range(UCC):
                    nc.sync.dma_start(LC1[s][:], d["LC1T"][s * 128:(s + 1) * 128, :])
                # process products in halves of 512 to bound live E tiles
                for (n0, nn) in _nt(PC):
                    E = [[None] * UCC for _ in range(H)]
                    for h in range(H):
                        pb = bcast(adrow[h], n0, nn)
                        for s in range(UCC):
                            A = cA.tile([128, 512], F32, name="c2vA_", tag="c2vA_",
                                        bufs=2)
                            nc.scalar.activation(A[:, 0:nn], pb[:], Act.Prelu,
                                                 bias=asu[s][:, h:h + 1], alpha=0.2)
                            A2 = cA.tile([128, 512], F32, name="c2vA2", tag="c2vA2",
                                         bufs=2)
                            nc.vector.tensor_add(A2[:, 0:nn], A[:, 0:nn],
                                                 LC1[s][:, n0:n0 + nn])
                            Et = cA.tile([128, 512], F32R, name="c2vE", tag="c2vE",
                                         bufs=2 * UCC)
                            nc.scalar.activation(Et[:, 0:nn], A2[:, 0:nn], Act.Exp)
                            E[h][s] = Et
                    rcps = []
                    for h in range(H):
                        pd = pdn([1, nn])
                        for s in range(UCC):
                            nc.tensor.matmul(pd[:], ones_r[:, 0:1], E[h][s][:, 0:nn],
                                             start=(s == 0), stop=(s == UCC - 1))
                        den2 = cA.tile([1, 512], F32, name="c2vden2", tag="c2vden2",
                                       bufs=2)
                        nc.scalar.activation(den2[:, 0:nn], pd[:], Act.Identity,
                                             bias=eps1[:])
                        rcp = cA.tile([1, 512], F32, name="c2vrcp", tag="c2vrcp",
                                      bufs=2)
                        nc.vector.reciprocal(rcp[:, 0:nn], den2[:, 0:nn])
                        rcps.append(rcp)
                    # aggregate + normalize + bias + relu into gatT[:, n0:n0+nn]
                    for h in range(H):
                        rb = bcast_sb(rcps[h], 0, nn, cA, "c2vrb")
                        for mi in range(DC):
                            po = pmm([128, nn])
                            for s in range(UCC):
                                nc.tensor.matmul(
                                    po[:],
                                    xsu[s][:, h * D + mi * 128:h * D + (mi + 1) * 128],
                                    E[h][s][:, 0:nn],
                                    start=(s == 0), stop=(s == UCC - 1))
                            tmp = cA.tile([128, 512], F32, name="c2vnr", tag="c2vnr",
                                          bufs=3)
                            nc.vector.tensor_mul(tmp[:, 0:nn], po[:], rb[:])
                            nc.scalar.activation(
                                gatT[h * DC + mi][:, n0:n0 + nn], tmp[:, 0:nn],
                                Act.Relu, bias=gb[0][:, h * DC + mi, :])
            dump("c2v_gat", gatT, PC)
            with tc.tile_pool(name="c2vB", bufs=1) as cB:
                r1 = [cB.tile([128, PC], F32R, name=f"c2vr1{i}") for i in range(DC)]
                ft_gemm(d["g0_ltW"], D2, D,
                        lambda ki, n0, nn: gatT[ki][:, n0:n0 + nn], PC,
                        ev_bias(r1, Act.Relu, ltb[0]))
                x2 = [cB.tile([128, PC], F32R, name=f"c2vx2{i}") for i in range(DC)]
                ft_gemm(d["g0_ffW"], D, D,
                        lambda ki, n0, nn: r1[ki][:, n0:n0 + nn], PC,
                        ev_bias(x2, Act.Identity, ffb[0]))
                ft_gemm(d["g0_pcW"], D2, D,
                        lambda ki, n0, nn: (pr0T[ki] if ki < DC
                                            else x2[ki - DC])[:, n0:n0 + nn], PC,
                        ev_bias(pr1T, Act.Relu, pcb[0]))
    for mi in range(DC):
        nc.sync.dma_start(d["pr1T"][mi * 128:(mi + 1) * 128, :],
                          pr1T[mi][:].bitcast(F32))
    dump("pr1T", pr1T, PC)

    # ============ stages 3-4: v2c / p2p (sharded src, AllReduce) ============
    def gat_sharded(gi, dstT, d_LC, ND, outT, name):
        with tc.tile_pool(name=f"{name}P", bufs=1) as P:
            asp = scores_nm(P, pr1T, wsa[gi], PCC, f"{name}as")
            adrow = scores_fr(P, dstT, wda[gi], ND, f"{name}ad")
            LCs = [P.tile([128, ND], F32, name=f"{name}lc{di}", tag=f"{name}lc",
                          bufs=PCC) for di in range(PCC)]
            for di in range(PCC):
                nc.sync.dma_start(LCs[di][:], d_LC[di * 128:(di + 1) * 128, :])
            den = [P.tile([1, ND], F32, name=f"{name}den{h}") for h in range(H)]
            num = [P.tile([128, ND], F32, name=f"{name}nm{t}", tag=f"{name}nm",
                          bufs=D2C) for t in range(D2C)]
            # per head: stage Ws half, xs, E, denominator-partial, numerator-partial
            for h in range(H):
                WsS = P.tile([128, DC, D], F32R, name=f"{name}ws", tag=f"{name}ws",
                             bufs=1)
                nc.sync.dma_start(
                    WsS[:], d[f"g{gi}_Ws"].bitcast(F32R)
                    .rearrange("(a p) m -> p a m", p=128)[:, :, h * D:(h + 1) * D])
                xs = [P.tile([128, D], F32R, name=f"{name}xs{di}", tag=f"{name}xs",
                             bufs=PCC) for di in range(PCC)]
                for di in range(PCC):
                    for (n0, nn) in _nt(D):
                        po = pmm([128, nn])
                        for ki in range(DC):
                            nc.tensor.matmul(po[:],
                                             pr1T[ki][:, di * 128:(di + 1) * 128],
                                             WsS[:, ki, n0:n0 + nn],
                                             start=(ki == 0), stop=(ki == DC - 1))
                        nc.scalar.copy(xs[di][:, n0:n0 + nn], po[:])
                pb = bcast(adrow[h], 0, ND)
                E = []
                for di in range(PCC):
                    A = P.tile([128, ND], F32, name=f"{name}A", tag=f"{name}A", bufs=2)
                    nc.scalar.activation(A[:], pb[:], Act.Prelu,
                                         bias=asp[di][:, h:h + 1], alpha=0.2)
                    A2 = P.tile([128, ND], F32, name=f"{name}A2", tag=f"{name}A2",
                                bufs=2)
                    nc.vector.tensor_add(A2[:], A[:], LCs[di][:])
                    Et = P.tile([128, ND], F32R, name=f"{name}E", tag=f"{name}E",
                                bufs=PCC)
                    nc.scalar.activation(Et[:], A2[:], Act.Exp)
                    E.append(Et)
                pd = pdn([1, ND])
                for di in range(PCC):
                    nc.tensor.matmul(pd[:], ones_r[:, 0:1], E[di][:],
                                     start=(di == 0), stop=(di == PCC - 1))
                nc.scalar.copy(den[h][:], pd[:])
                for mi in range(DC):
                    po = pmm([128, ND])
                    for di in range(PCC):
                        nc.tensor.matmul(po[:],
                                         xs[di][:, mi * 128:(mi + 1) * 128],
                                         E[di][:], start=(di == 0),
                                         stop=(di == PCC - 1))
                    nc.scalar.copy(num[h * DC + mi][:], po[:])
            # AllReduce numerator+denominator
            bin_ = drm.tile([D2 + H, ND], F32, name=f"{name}ari")
            bout = drm.tile([D2 + H, ND], F32, name=f"{name}aro", addr_space="Shared")
            for t in range(D2C):
                nc.sync.dma_start(bin_[t * 128:(t + 1) * 128, :], num[t][:])
            for h in range(H):
                nc.sync.dma_start(bin_[D2 + h:D2 + h + 1, :], den[h][:])
            nc.gpsimd.collective_compute(
                "AllReduce", mybir.AluOpType.add,
                replica_groups=[list(range(NCORE))],
                ins=[bin_[:].opt()], outs=[bout[:].opt()])
            numr = [P.tile([128, ND], F32, name=f"{name}nr{t}", tag=f"{name}nm",
                           bufs=D2C) for t in range(D2C)]
            for t in range(D2C):
                nc.sync.dma_start(numr[t][:], bout[t * 128:(t + 1) * 128, :])
            rcps = []
            for h in range(H):
                denr = P.tile([1, ND], F32, name=f"{name}denr{h}")
                nc.sync.dma_start(denr[:], bout[D2 + h:D2 + h + 1, :])
                den2 = P.tile([1, ND], F32, name=f"{name}den2{h}")
                nc.scalar.activation(den2[:], denr[:], Act.Identity,
                                     bias=eps1[:])
                rcp = P.tile([1, ND], F32, name=f"{name}rcp{h}")
                nc.vector.reciprocal(rcp[:], den2[:])
                rcps.append(rcp)
            gat = [P.tile([128, ND], F32R, name=f"{name}g{t}", tag=f"{name}g",
                          bufs=D2C) for t in range(D2C)]
            for h in range(H):
                rb = bcast_sb(rcps[h], 0, ND, P, f"{name}rb")
                for mi in range(DC):
                    tmp = P.tile([128, ND], F32, name=f"{name}t", tag=f"{name}t",
                                 bufs=2)
                    nc.vector.tensor_mul(tmp[:], numr[h * DC + mi][:], rb[:])
                    nc.scalar.activation(gat[h * DC + mi][:], tmp[:], Act.Relu,
                                         bias=gb[gi][:, h * DC + mi, :])
            dump(f"{name}_gat", gat, ND)
            r1 = [P.tile([128, ND], F32R, name=f"{name}r1{i}") for i in range(DC)]
            ft_gemm(d[f"g{gi}_ltW"], D2, D,
                    lambda ki, n0, nn: gat[ki][:, n0:n0 + nn], ND,
                    ev_bias(r1, Act.Relu, ltb[gi]))
            x2 = [P.tile([128, ND], F32R, name=f"{name}x2{i}") for i in range(DC)]
            ft_gemm(d[f"g{gi}_ffW"], D, D,
                    lambda ki, n0, nn: r1[ki][:, n0:n0 + nn], ND,
                    ev_bias(x2, Act.Identity, ffb[gi]))
            ft_gemm(d[f"g{gi}_pcW"], D2, D,
                    lambda ki, n0, nn: (dstT[ki] if ki < DC
                                        else x2[ki - DC])[:, n0:n0 + nn], ND,
                    ev_bias(outT, Act.Relu, pcb[gi]))

    gat_sharded(1, u0T, d["LC2"], NU, u1T, "v2c")
    for mi in range(DC):
        nc.sync.dma_start(d["u1T"][mi * 128:(mi + 1) * 128, :],
                          u1T[mi][:].bitcast(F32))
    gat_sharded(2, pe0T, d["LC3"], NK, pe1T, "p2p")
    dump("u1T", u1T, NU)
    dump("pe1T", pe1T, NK)

    # ============ stage 5: head ============
    with tc.tile_pool(name="head", bufs=1) as hp:
        peaT = [hp.tile([128, NK], F32, name=f"peaT{i}") for i in range(DC)]
        ft_gemm(d["fc0a"], D, D, lambda ki, n0, nn: pe1T[ki][:, n0:n0 + nn], NK,
                ev_bias(peaT, Act.Identity, b0S))
        fc0bS = stage_km(hp, d["fc0b"], D, D, "fc0bS")
        ubn = [hp.tile([128, D], F32R, name=f"ubn{s}") for s in range(UCC)]
        for s in range(UCC):
            for (n0, nn) in _nt(D):
                po = pmm([128, nn])
                for ki in range(DC):
                    nc.tensor.matmul(po[:], u1T[ki][:, s * 128:(s + 1) * 128],
                                     fc0bS[:, ki, n0:n0 + nn],
                                     start=(ki == 0), stop=(ki == DC - 1))
                nc.scalar.copy(ubn[s][:, n0:n0 + nn], po[:])
        SselS = [hp.tile([128, UC], F32R, name=f"ssel{s}") for s in range(UCC)]
        for s in range(UCC):
            nc.sync.dma_start(SselS[s][:],
                              d["Ssel"][s * 128:(s + 1) * 128, :].bitcast(F32R))
        ubsl = [hp.tile([128, UC], F32, name=f"ubsl{mi}") for mi in range(DC)]
        for mi in range(DC):
            po = pmm([128, UC])
            for s in range(UCC):
                nc.tensor.matmul(po[:], ubn[s][:, mi * 128:(mi + 1) * 128],
                                 SselS[s][:], start=(s == 0), stop=(s == UCC - 1))
            nc.scalar.copy(ubsl[mi][:], po[:])
        dump("peaT", peaT, NK)
        dump("ubsl", ubsl, UC)
        fc1S = stage_km(hp, d["fc1W"], D, 384, "fc1S")
        fc2S = stage_km(hp, d["fc2W"], 384, 128, "fc2S")
        fc3S = hp.tile([128, 1], F32R, name="fc3S")
        nc.sync.dma_start(fc3S[:], d["fc3W"].bitcast(F32R))
        for grp in range(NPAIR // 512):
            h0 = hp.tile([128, DC, 512], F32R, name="h0g", tag="h0g", bufs=2)
            for ci in range(DC):
                for il in range(4):
                    nc.vector.tensor_scalar(
                        h0[:, ci, il * 128:(il + 1) * 128], peaT[ci][:],
                        ubsl[ci][:, grp * 4 + il:grp * 4 + il + 1], 0.0,
                        AluOpType.add, AluOpType.max)
            h1 = hp.tile([128, 3, 512], F32R, name="h1g", tag="h1g", bufs=2)
            for mi in range(3):
                po = pmm([128, 512])
                for ki in range(DC):
                    nc.tensor.matmul(po[:], fc1S[:, ki, mi * 128:(mi + 1) * 128],
                                     h0[:, ki, :], start=(ki == 0),
                                     stop=(ki == DC - 1))
                nc.scalar.activation(h1[:, mi, :], po[:], Act.Relu,
                                     bias=fc1bS[:, mi, :])
            h2 = hp.tile([128, 512], F32R, name="h2g", tag="h2g", bufs=2)
            po = pmm([128, 512])
            for ki in range(3):
                nc.tensor.matmul(po[:], fc2S[:, ki, :], h1[:, ki, :],
                                 start=(ki == 0), stop=(ki == 2))
            nc.scalar.activation(h2[:], po[:], Act.Relu, bias=fc2bS[:, 0, :])
            pd = pdn([1, 512])
            nc.tensor.matmul(pd[:], fc3S[:], h2[:], start=True, stop=True)
            yt = hp.tile([1, 512], F32, name="yt", tag="yt", bufs=2)
            nc.scalar.activation(yt[:], pd[:], Act.Sigmoid, bias=fc3bS[0:1, :])
            nc.sync.dma_start(d["y"][:, grp * 512:(grp + 1) * 512], yt[:])
    ctx.close()


# ================= host side =================

def _prep(inputs):
    """Host-side preprocessing: padding, transposes, BN folding, edge->mask."""
    def deep(v):
        if isinstance(v, dict):
            return {k: deep(x) for k, x in v.items()}
        return np.asarray(v)

    inputs = deep(inputs)
    p = inputs["params"]
    f32 = np.float32

    def padK(W):  # [F_IN, M] -> [FP, M]
        W = np.asarray(W, f32)
        return np.concatenate([W, np.zeros((FP - F_IN, W.shape[1]), f32)], 0)

    def padKT(X):  # [N, F_IN] -> [FP, N]
        X = np.asarray(X, f32).T
        return np.concatenate([X, np.zeros((FP - F_IN, X.shape[1]), f32)], 0)

    rep = {"c_ones": np.ones((128, 128), f32)}
    rep["ufT"] = padKT(inputs["user_features"])
    rep["kfT"] = padKT(inputs["persona_features"])
    rep["W_user"] = padK(p["W_user"])
    rep["W_prod"] = padK(p["W_prod"])
    rep["W_pers"] = padK(p["W_pers"])
    rep["b_user"] = np.asarray(p["b_user"], f32).reshape(D, 1)
    rep["b_prod"] = np.asarray(p["b_prod"], f32).reshape(D, 1)
    rep["b_pers"] = np.asarray(p["b_pers"], f32).reshape(D, 1)
    for g, key in enumerate(("c2v", "v2c", "p2p")):
        gp = p[key]
        Ws = np.asarray(gp["Ws"], f32)
        Wd = np.asarray(gp["Wd"], f32)
        att_s = np.asarray(gp["att_s"], f32)
        att_d = np.asarray(gp["att_d"], f32)
        rep[f"g{g}_Ws"] = Ws
        rep[f"g{g}_Wsatt"] = np.stack(
            [Ws[:, h * D:(h + 1) * D] @ att_s[h] for h in range(H)], 1)
        rep[f"g{g}_Wdatt"] = np.stack(
            [Wd[:, h * D:(h + 1) * D] @ att_d[h] for h in range(H)], 1)
        rep[f"g{g}_gbias"] = np.asarray(gp["bias"], f32).reshape(D2, 1)
        rep[f"g{g}_ltW"] = np.asarray(gp["lt_W"], f32)
        rep[f"g{g}_ltb"] = np.asarray(gp["lt_b"], f32).reshape(D, 1)
        rep[f"g{g}_ffW"] = np.asarray(gp["ff_W"], f32)
        rep[f"g{g}_ffb"] = np.asarray(gp["ff_b"], f32).reshape(D, 1)
        rep[f"g{g}_pcW"] = np.asarray(gp["pc_W"], f32)
        rep[f"g{g}_pcb"] = np.asarray(gp["pc_b"], f32).reshape(D, 1)
    s0 = f32(1.0 / np.sqrt(1.0 + 1e-5))
    g0 = np.asarray(p["bn0_g"], f32) * s0
    g1 = np.asarray(p["bn1_g"], f32) * s0
    g2 = np.asarray(p["bn2_g"], f32) * s0
    fc0 = np.asarray(p["fc0_W"], f32)
    rep["fc0a"] = fc0[:D] * g0[None, :]
    rep["fc0b"] = fc0[D:] * g0[None, :]
    rep["b0"] = (np.asarray(p["fc0_b"], f32) * g0
                 + np.asarray(p["bn0_b"], f32)).reshape(D, 1)
    rep["fc1W"] = np.asarray(p["fc1_W"], f32) * g1[None, :]
    rep["fc1b"] = (np.asarray(p["fc1_b"], f32) * g1
                   + np.asarray(p["bn1_b"], f32)).reshape(384, 1)
    rep["fc2W"] = np.asarray(p["fc2_W"], f32) * g2[None, :]
    rep["fc2b"] = (np.asarray(p["fc2_b"], f32) * g2
                   + np.asarray(p["bn2_b"], f32)).reshape(128, 1)
    rep["fc3W"] = np.asarray(p["fc3_W"], f32).reshape(128, 1)
    rep["fc3b"] = np.asarray(p["fc3_b"], f32).reshape(1, 1)

    # edge lists -> dense log-count masks
    ei = np.asarray(inputs["edge_indices"]).astype(np.int64)
    ei0 = ei[0] - ei[0].min()       # product (dst of c2v)
    ei1 = ei[1]                     # user
    C1 = np.zeros((NP, NU), f32)
    np.add.at(C1, (ei0, ei1), f32(1.0))
    pp = np.asarray(inputs["persona_prod_edge_ind"]).astype(np.int64)
    pp0 = pp[0]                     # product (src of p2p)
    pp1 = pp[1] - pp[1].min()       # persona (dst)
    C2 = np.zeros((NK, NP), f32)
    np.add.at(C2, (pp1, pp0), f32(1.0))

    def logmask(C):
        out = np.full(C.shape, MASK, f32)
        nz = C > 0
        out[nz] = np.log(C[nz])
        return out

    LC1 = logmask(C1)               # [NP, NU]
    LC3 = logmask(C2.T)             # [NP, NK]
    pfT = padKT(inputs["product_features"])

    in_maps = []
    for c in range(NCORE):
        sl = slice(c * PC, (c + 1) * PC)
        m = dict(rep)
        m["pfT"] = np.ascontiguousarray(pfT[:, sl])
        m["LC1T"] = np.ascontiguousarray(LC1[sl].T)
        m["LC2"] = np.ascontiguousarray(LC1[sl])
        m["LC3"] = np.ascontiguousarray(LC3[sl])
        S = np.zeros((NU, UC), f32)
        S[np.arange(c * UC, (c + 1) * UC), np.arange(UC)] = 1.0
        m["Ssel"] = S
        in_maps.append(m)
    return in_maps


_CACHE = {}
TRACE = False
DEBUG = False
LAST = {}


def kernel(**inputs):
    key = ("nc", DEBUG)
    if key not in _CACHE:
        _CACHE[key] = _build(debug=DEBUG)
    nc = _CACHE[key]
    in_maps = _prep(inputs)
    res = run_bass_kernel_spmd(nc, in_maps, core_ids=list(range(NCORE)),
                               trace=TRACE)
    LAST["res"] = res
    r = res.results
    x = np.concatenate([r[c]["y"][0] for c in range(NCORE)]).reshape(NU * NK, 1)
    u = np.ascontiguousarray(r[0]["u1T"].T)
    pr = np.concatenate([r[c]["pr1T"].T for c in range(NCORE)], 0)
    return x.astype(np.float32), u.astype(np.float32), pr.astype(np.float32)


# revision 16
# speedup vs baseline: 1.0244x; 1.0244x over previous
"""Trainium2 Bass kernel for nn_GATPolicy (3x bipartite GATConv + cartesian MLP).

8 NeuronCores, SPMD. Key restructurings (all exact up to fp rounding):
  * Products (8192) row-sharded 1024/core; users/personas replicated;
    head sharded 32 users/core.
  * Graph edges -> dense log-count masks on the host:
    LC[dst,src] = ln(multiplicity), -200 if absent.  Segment softmax becomes
    dense masked softmax (duplicate edges share identical logits so they fold
    into ln(count); exp(x-200) underflows to exact +0).  Aggregation becomes
    out = softmax(lrelu(as+ad)+LC) @ xs - dense matmuls.
  * xd is only needed through per-node scores: ad = x_dst @ (Wd@att_d),
    folded on the host.  edge_attr == 0 so the We path vanishes.
  * Activations feature-major (T[feat, node]); weights [K,M] used directly as
    the stationary matmul operand; biases fused into PSUM-evict activations.
  * v2c/p2p contract over the sharded product dim: one AllReduce per net of
    numerator[1536,Nd]+denominator[2,Nd]; softmax normalization after.
  * Head fc0 factorized over the cartesian product; eval-BN folded into
    weights host-side.
  * Matmuls in float32r (tf32-class, ~1.6e-4), fp32 accumulate.
"""
import numpy as np

import concourse.bass as bass
import concourse.tile as tile
from concourse import bacc, mybir
from concourse.alu_op_type import AluOpType
from concourse.bass_utils import run_bass_kernel_spmd

dt = mybir.dt
F32 = dt.float32
F32R = dt.float32r
Act = mybir.ActivationFunctionType

D = 768
H = 2
D2 = 1536
NU = 256
NP = 8192
NK = 128
F_IN = 3551
FP = 3584            # F_IN zero-padded to 28*128
NCORE = 8
PC = NP // NCORE     # 1024 products/core
UC = NU // NCORE     # 32 users/core
NPAIR = UC * NK      # 4096 pairs/core
DC = D // 128        # 6
D2C = D2 // 128      # 12
KC = FP // 128       # 28
PCC = PC // 128      # 8
UCC = NU // 128      # 2
MASK = np.float32(-200.0)


def _nt(N, nt=512):
    out, n0 = [], 0
    while n0 < N:
        nn = min(nt, N - n0)
        out.append((n0, nn))
        n0 += nn
    return out


def _build(debug=False):
    nc = bacc.Bacc("TRN2", target_bir_lowering=False, debug=False,
                   num_devices=NCORE)

    def inp(name, shape):
        return nc.dram_tensor(name, list(shape), F32, kind="ExternalInput").ap()

    def outp(name, shape):
        return nc.dram_tensor(name, list(shape), F32, kind="ExternalOutput").ap()

    d = {}
    d["ones"] = inp("c_ones", [128, 128])
    d["ufT"] = inp("ufT", [FP, NU])
    d["kfT"] = inp("kfT", [FP, NK])
    for nm in ("W_user", "W_prod", "W_pers"):
        d[nm] = inp(nm, [FP, D])
    for nm in ("b_user", "b_prod", "b_pers"):
        d[nm] = inp(nm, [D, 1])
    for g in range(3):
        d[f"g{g}_Ws"] = inp(f"g{g}_Ws", [D, D2])
        d[f"g{g}_Wsatt"] = inp(f"g{g}_Wsatt", [D, H])
        d[f"g{g}_Wdatt"] = inp(f"g{g}_Wdatt", [D, H])
        d[f"g{g}_gbias"] = inp(f"g{g}_gbias", [D2, 1])
        d[f"g{g}_ltW"] = inp(f"g{g}_ltW", [D2, D])
        d[f"g{g}_ltb"] = inp(f"g{g}_ltb", [D, 1])
        d[f"g{g}_ffW"] = inp(f"g{g}_ffW", [D, D])
        d[f"g{g}_ffb"] = inp(f"g{g}_ffb", [D, 1])
        d[f"g{g}_pcW"] = inp(f"g{g}_pcW", [D2, D])
        d[f"g{g}_pcb"] = inp(f"g{g}_pcb", [D, 1])
    d["fc0a"] = inp("fc0a", [D, D])
    d["b0"] = inp("b0", [D, 1])
    d["fc0b"] = inp("fc0b", [D, D])
    d["fc1W"] = inp("fc1W", [D, 384])
    d["fc1b"] = inp("fc1b", [384, 1])
    d["fc2W"] = inp("fc2W", [384, 128])
    d["fc2b"] = inp("fc2b", [128, 1])
    d["fc3W"] = inp("fc3W", [128, 1])
    d["fc3b"] = inp("fc3b", [1, 1])
    d["pfT"] = inp("pfT", [FP, PC])
    d["LC1T"] = inp("LC1T", [NU, PC])
    d["LC2"] = inp("LC2", [PC, NU])
    d["LC3"] = inp("LC3", [PC, NK])
    d["Ssel"] = inp("Ssel", [NU, UC])
    d["y"] = outp("y", [1, NPAIR])
    d["u1T"] = outp("u1T", [D, NU])
    d["pr1T"] = outp("pr1T", [D, PC])

    with tile.TileContext(nc) as tc:
        _emit(nc, tc, d, debug)
    nc.compile()
    return nc


def _emit(nc, tc, d, debug):
    from contextlib import ExitStack
    ctx = ExitStack()
    glob = ctx.enter_context(tc.tile_pool(name="glob", bufs=1))
    wk = ctx.enter_context(tc.tile_pool(name="wk", bufs=16))
    ps = ctx.enter_context(tc.tile_pool(name="ps", bufs=1, space="PSUM"))
    drm = ctx.enter_context(tc.tile_pool(name="drm", bufs=1, space="DRAM"))

    def pmm(shape):
        return ps.tile(shape, F32, name="mmp", tag="mm", bufs=3)

    def pbc(shape):
        return ps.tile(shape, F32, name="bcp", tag="bc", bufs=2)

    def pdn(shape):
        return ps.tile(shape, F32, name="dnp", tag="dn", bufs=1)

    def dump(name, tiles, N, parts=128):
        if not debug:
            return
        ap = nc.dram_tensor("dbg_" + name, [len(tiles) * parts, N], F32,
                            kind="ExternalOutput").ap()
        for i, t in enumerate(tiles):
            src = t[0:parts, 0:N]
            if src.dtype == F32R:
                src = src.bitcast(F32)
            nc.sync.dma_start(ap[i * parts:(i + 1) * parts, :], src)

    def stage_bias(dap, M, name):
        mc = -(-M // 128)
        t = glob.tile([128, mc, 1], F32, name="b_" + name)
        nc.sync.dma_start(t[:], dap.rearrange("(a p) o -> p a o", p=128))
        return t

    def stage_km(pool, dap, K, M, name, wdt=F32R, tag="", bufs=0):
        """Stage a [K, M] dram weight as per-ki [128, M] tiles (one DMA each,
        so consumers start as soon as their chunk lands)."""
        kc = K // 128
        ts = [pool.tile([128, M], wdt, name=f"{name}_{ki}", tag=tag or name,
                        bufs=bufs or kc) for ki in range(kc)]
        for ki in range(kc):
            s = dap[ki * 128:(ki + 1) * 128, :]
            nc.sync.dma_start(ts[ki][:], s.bitcast(wdt) if wdt != F32 else s)
        return ts

    def ft_gemm(Wd, K, M, rhs_at, N, evict, live=2):
        """out chunks [mi, n0:nn] = W[K,M].T @ X[K,N], W streamed from DRAM."""
        kc = K // 128
        tiles = _nt(N)
        for mi in range(M // 128):
            for gs in range(0, len(tiles), live):
                grp = tiles[gs:gs + live]
                pos = [pmm([128, nn]) for (_, nn) in grp]
                for ki in range(kc):
                    w = wk.tile([128, 128], F32R, name="wchunk")
                    nc.sync.dma_start(
                        w[:], Wd[ki * 128:(ki + 1) * 128,
                                 mi * 128:(mi + 1) * 128].bitcast(F32R))
                    for po, (n0, nn) in zip(pos, grp):
                        nc.tensor.matmul(po[:], w[:], rhs_at(ki, n0, nn),
                                         start=(ki == 0), stop=(ki == kc - 1))
                for po, (n0, nn) in zip(pos, grp):
                    evict(po, mi, n0, nn)

    def ev_bias(out_tiles, func, biasS, coff=0):
        def ev(po, mi, n0, nn):
            nc.scalar.activation(out_tiles[mi][:, coff + n0:coff + n0 + nn],
                                 po[:], func, bias=biasS[:, mi, :])
        return ev

    def scores_nm(pool, xT, wattS, nchunks, name):
        """as[node,h] node-major per 128-chunk: x @ (W@att)."""
        out = []
        for s in range(nchunks):
            po = pmm([128, H])
            for ki in range(DC):
                nc.tensor.matmul(po[:], xT[ki][:, s * 128:(s + 1) * 128],
                                 wattS[ki][:], start=(ki == 0), stop=(ki == DC - 1))
            t = pool.tile([128, H], F32, name=f"{name}{s}", tag=name, bufs=nchunks)
            nc.scalar.copy(t[:], po[:])
            out.append(t)
        return out

    def scores_fr(pool, xT, wattS, N, name):
        """ad[h][node] feature-rows: (W@att).T @ xT, one [1,N] tile per head."""
        ts = [pool.tile([1, N], F32, name=f"{name}{h}") for h in range(H)]
        scores_fr_into(ts, xT, wattS, N)
        return ts

    def scores_fr_into(ts, xT, wattS, N):
        for h in range(H):
            for (n0, nn) in _nt(N):
                pd = pdn([1, nn])
                for ki in range(DC):
                    nc.tensor.matmul(pd[:], wattS[ki][:, h:h + 1],
                                     xT[ki][:, n0:n0 + nn],
                                     start=(ki == 0), stop=(ki == DC - 1))
                nc.scalar.copy(ts[h][:, n0:n0 + nn], pd[:])

    def bcast(rowt, n0, nn):
        """Broadcast rowt[0, n0:n0+nn] to 128 partitions (K=1 ones-matmul)."""
        pb = pbc([128, nn])
        for (m0, mm) in _nt(nn):
            nc.tensor.matmul(pb[:, m0:m0 + mm], ones_f[0:1, :],
                             rowt[0:1, n0 + m0:n0 + m0 + mm],
                             start=True, stop=True)
        return pb

    def bcast_sb(rowt, n0, nn, pool, tag):
        """bcast + evict to SBUF (for use as a DVE tensor_tensor operand)."""
        pb = bcast(rowt, n0, nn)
        t = pool.tile([128, nn], F32, name=tag, tag=tag, bufs=2)
        nc.scalar.copy(t[:], pb[:])
        return t

    # ---------------- constants
    ones_f = glob.tile([128, 128], F32, name="ones_f")
    nc.sync.dma_start(ones_f[:], d["ones"])
    eps1 = glob.tile([1, 1], F32, name="eps1")
    nc.vector.memset(eps1[:], 1e-16)
    ones_r = glob.tile([128, 128], F32R, name="ones_r")
    nc.sync.dma_start(ones_r[:], d["ones"].bitcast(F32R))
    bias_u = stage_bias(d["b_user"], D, "bu")
    bias_p = stage_bias(d["b_prod"], D, "bp")
    bias_k = stage_bias(d["b_pers"], D, "bk")
    gb, ltb, ffb, pcb, wsa, wda = [], [], [], [], [], []
    for g in range(3):
        gb.append(stage_bias(d[f"g{g}_gbias"], D2, f"g{g}gb"))
        ltb.append(stage_bias(d[f"g{g}_ltb"], D, f"g{g}lb"))
        ffb.append(stage_bias(d[f"g{g}_ffb"], D, f"g{g}fb"))
        pcb.append(stage_bias(d[f"g{g}_pcb"], D, f"g{g}pb"))
        wsa.append(stage_km(glob, d[f"g{g}_Wsatt"], D, H, f"g{g}wsa"))
        wda.append(stage_km(glob, d[f"g{g}_Wdatt"], D, H, f"g{g}wda"))
    b0S = stage_bias(d["b0"], D, "b0")
    fc1bS = stage_bias(d["fc1b"], 384, "fc1b")
    fc2bS = stage_bias(d["fc2b"], 128, "fc2b")
    fc3bS = glob.tile([1, 1], F32, name="fc3bS")
    nc.sync.dma_start(fc3bS[:], d["fc3b"])

    # persistent activations
    u0T = [glob.tile([128, NU], F32R, name=f"u0T{i}") for i in range(DC)]
    pe0T = [glob.tile([128, NK], F32R, name=f"pe0T{i}") for i in range(DC)]
    pr1T = [glob.tile([128, PC], F32R, name=f"pr1T{i}") for i in range(DC)]
    u1T = [glob.tile([128, NU], F32R, name=f"u1T{i}") for i in range(DC)]
    pe1T = [glob.tile([128, NK], F32R, name=f"pe1T{i}") for i in range(DC)]

    # ============ stage 1: embeddings ============
    with tc.tile_pool(name="pr0p", bufs=1) as pr0p:
        pr0T = [pr0p.tile([128, PC], F32R, name=f"pr0T{i}") for i in range(DC)]
        LC1 = [pr0p.tile([128, PC], F32, name=f"c2vlc{s}") for s in range(UCC)]
        for s in range(UCC):
            nc.sync.dma_start(LC1[s][:], d["LC1T"][s * 128:(s + 1) * 128, :])
        with tc.tile_pool(name="embA", bufs=1) as embA:
            ufk = stage_km(embA, d["ufT"], FP, NU, "ufk")
            kfk = stage_km(embA, d["kfT"], FP, NK, "kfk")
            ft_gemm(d["W_user"], FP, D,
                    lambda ki, n0, nn: ufk[ki][:, n0:n0 + nn], NU,
                    ev_bias(u0T, Act.Identity, bias_u))
            ft_gemm(d["W_pers"], FP, D,
                    lambda ki, n0, nn: kfk[ki][:, n0:n0 + nn], NK,
                    ev_bias(pe0T, Act.Identity, bias_k))
        with tc.tile_pool(name="embB", bufs=1) as embB:
            for half in range(2):
                pfk = [embB.tile([128, PC // 2], F32R, name=f"pfk{half}_{ki}",
                                 tag="pfk", bufs=KC) for ki in range(KC)]
                for ki in range(KC):
                    nc.sync.dma_start(
                        pfk[ki][:],
                        d["pfT"][ki * 128:(ki + 1) * 128,
                                 half * 512:(half + 1) * 512].bitcast(F32R))
                ft_gemm(d["W_prod"], FP, D,
                        lambda ki, n0, nn: pfk[ki][:, n0:n0 + nn], PC // 2,
                        ev_bias(pr0T, Act.Identity, bias_p, coff=half * 512))
        dump("u0T", u0T, NU)
        dump("pe0T", pe0T, NK)
        dump("pr0T", pr0T, PC)

        # ============ stage 2: c2v (src=user, dst=product shard; local softmax)
        with tc.tile_pool(name="c2vT", bufs=1) as c2vT:
            gatT = [c2vT.tile([128, PC], F32R, name=f"c2vg{i}", tag="c2vg",
                              bufs=D2C) for i in range(D2C)]
            with tc.tile_pool(name="c2vA", bufs=1) as cA:
                asu = scores_nm(cA, u0T, wsa[0], UCC, "c2v_as")
                adrow = scores_fr(cA, pr0T, wda[0], PC, "c2v_ad")
                # xs node-major [256 x 1536] (weights streamed once)
                xsu = [cA.tile([128, D2], F32R, name=f"c2vxs{s}") for s in range(UCC)]
                for (n0, nn) in _nt(D2):
                    pos = [pmm([128, nn]) for _ in range(UCC)]
                    for ki in range(DC):
                        w = wk.tile([128, 512], F32R, name="wchunk2", tag="wchunk2",
                                    bufs=6)
                        nc.sync.dma_start(
                            w[:, 0:nn],
                            d["g0_Ws"][ki * 128:(ki + 1) * 128, n0:n0 + nn].bitcast(F32R))
                        for s in range(UCC):
                            nc.tensor.matmul(pos[s][:], u0T[ki][:, s * 128:(s + 1) * 128],
                                             w[:, 0:nn], start=(ki == 0),
                                             stop=(ki == DC - 1))
                    for s in range(UCC):
                        nc.scalar.copy(xsu[s][:, n0:n0 + nn], pos[s][:])
                # process products in halves of 512 to bound live E tiles
                for (n0, nn) in _nt(PC):
                    E = [[None] * UCC for _ in range(H)]
                    for h in range(H):
                        pb = bcast(adrow[h], n0, nn)
                        for s in range(UCC):
                            A = cA.tile([128, 512], F32, name="c2vA_", tag="c2vA_",
                                        bufs=2)
                            nc.scalar.activation(A[:, 0:nn], pb[:], Act.Prelu,
                                                 bias=asu[s][:, h:h + 1], alpha=0.2)
                            A2 = cA.tile([128, 512], F32, name="c2vA2", tag="c2vA2",
                                         bufs=2)
                            nc.vector.tensor_add(A2[:, 0:nn], A[:, 0:nn],
                                                 LC1[s][:, n0:n0 + nn])
                            Et = cA.tile([128, 512], F32R, name="c2vE", tag="c2vE",
                                         bufs=2 * UCC)
                            nc.scalar.activation(Et[:, 0:nn], A2[:, 0:nn], Act.Exp)
                            E[h][s] = Et
                    rcps = []
                    for h in range(H):
                        pd = pdn([1, nn])
                        for s in range(UCC):
                            nc.tensor.matmul(pd[:], ones_r[:, 0:1], E[h][s][:, 0:nn],
                                             start=(s == 0), stop=(s == UCC - 1))
                        den2 = cA.tile([1, 512], F32, name="c2vden2", tag="c2vden2",
                                       bufs=2)
                        nc.scalar.activation(den2[:, 0:nn], pd[:], Act.Identity,
                                             bias=eps1[:])
                        rcp = cA.tile([1, 512], F32, name="c2vrcp", tag="c2vrcp",
                                      bufs=2)
                        nc.vector.reciprocal(rcp[:, 0:nn], den2[:, 0:nn])
                        rcps.append(rcp)
                    # aggregate + normalize + bias + relu into gatT[:, n0:n0+nn]
                    for h in range(H):
                        rb = bcast_sb(rcps[h], 0, nn, cA, "c2vrb")
                        for mi in range(DC):
                            po = pmm([128, nn])
                            for s in range(UCC):
                                nc.tensor.matmul(
                                    po[:],
                                    xsu[s][:, h * D + mi * 128:h * D + (mi + 1) * 128],
                                    E[h][s][:, 0:nn],
                                    start=(s == 0), stop=(s == UCC - 1))
                            tmp = cA.tile([128, 512], F32, name="c2vnr", tag="c2vnr",
                                          bufs=3)
                            nc.vector.tensor_mul(tmp[:, 0:nn], po[:], rb[:])
                            nc.scalar.activation(
                                gatT[h * DC + mi][:, n0:n0 + nn], tmp[:, 0:nn],
                                Act.Relu, bias=gb[0][:, h * DC + mi, :])
            dump("c2v_gat", gatT, PC)
            with tc.tile_pool(name="c2vB", bufs=1) as cB:
                r1 = [cB.tile([128, PC], F32R, name=f"c2vr1{i}") for i in range(DC)]
                ft_gemm(d["g0_ltW"], D2, D,
                        lambda ki, n0, nn: gatT[ki][:, n0:n0 + nn], PC,
                        ev_bias(r1, Act.Relu, ltb[0]))
                x2 = [cB.tile([128, PC], F32R, name=f"c2vx2{i}") for i in range(DC)]
                ft_gemm(d["g0_ffW"], D, D,
                        lambda ki, n0, nn: r1[ki][:, n0:n0 + nn], PC,
                        ev_bias(x2, Act.Identity, ffb[0]))
                ft_gemm(d["g0_pcW"], D2, D,
                        lambda ki, n0, nn: (pr0T[ki] if ki < DC
                                            else x2[ki - DC])[:, n0:n0 + nn], PC,
                        ev_bias(pr1T, Act.Relu, pcb[0]))
    for mi in range(DC):
        nc.sync.dma_start(d["pr1T"][mi * 128:(mi + 1) * 128, :],
                          pr1T[mi][:].bitcast(F32))
    dump("pr1T", pr1T, PC)

    # ============ stages 3-4: v2c / p2p (sharded src, AllReduce) ============
    def gat_sharded(P, gi, dstT, d_LC, ND, outT, name):
            asp = scores_nm(P, pr1T, wsa[gi], PCC, "sh_as")
            adrow = [P.tile([1, ND], F32, name=f"{name}ad{h}", tag=f"sh_ad{h}", bufs=1)
                      for h in range(H)]
            scores_fr_into(adrow, dstT, wda[gi], ND)
            LCs = [P.tile([128, ND], F32, name=f"{name}lc{di}", tag="sh_lc",
                          bufs=2 * PCC) for di in range(PCC)]
            for di in range(PCC):
                nc.sync.dma_start(LCs[di][:], d_LC[di * 128:(di + 1) * 128, :])
            den = [P.tile([1, ND], F32, name=f"{name}den{h}", tag=f"sh_den{h}",
                          bufs=1) for h in range(H)]
            num = [P.tile([128, ND], F32, name=f"{name}nm{t}", tag="sh_nm",
                          bufs=D2C + 4) for t in range(D2C)]
            # per head: stage Ws half, xs, E, denominator-partial, numerator-partial
            for h in range(H):
                WsS = [P.tile([128, D], F32R, name=f"{name}ws{ki}", tag="sh_ws",
                               bufs=DC) for ki in range(DC)]
                for ki in range(DC):
                    nc.sync.dma_start(
                        WsS[ki][:],
                        d[f"g{gi}_Ws"][ki * 128:(ki + 1) * 128,
                                       h * D:(h + 1) * D].bitcast(F32R))
                xs = [P.tile([128, D], F32R, name=f"{name}xs{di}", tag="sh_xs",
                             bufs=PCC) for di in range(PCC)]
                for di in range(PCC):
                    for (n0, nn) in _nt(D):
                        po = pmm([128, nn])
                        for ki in range(DC):
                            nc.tensor.matmul(po[:],
                                             pr1T[ki][:, di * 128:(di + 1) * 128],
                                             WsS[ki][:, n0:n0 + nn],
                                             start=(ki == 0), stop=(ki == DC - 1))
                        nc.scalar.copy(xs[di][:, n0:n0 + nn], po[:])
                pb = bcast(adrow[h], 0, ND)
                E = []
                for di in range(PCC):
                    A = P.tile([128, ND], F32, name=f"{name}A", tag="sh_A", bufs=2)
                    nc.scalar.activation(A[:], pb[:], Act.Prelu,
                                         bias=asp[di][:, h:h + 1], alpha=0.2)
                    A2 = P.tile([128, ND], F32, name=f"{name}A2", tag="sh_A2",
                                bufs=2)
                    nc.vector.tensor_add(A2[:], A[:], LCs[di][:])
                    Et = P.tile([128, ND], F32R, name=f"{name}E", tag="sh_E",
                                bufs=PCC)
                    nc.scalar.activation(Et[:], A2[:], Act.Exp)
                    E.append(Et)
                pd = pdn([1, ND])
                for di in range(PCC):
                    nc.tensor.matmul(pd[:], ones_r[:, 0:1], E[di][:],
                                     start=(di == 0), stop=(di == PCC - 1))
                nc.scalar.copy(den[h][:], pd[:])
                for mi in range(DC):
                    po = pmm([128, ND])
                    for di in range(PCC):
                        nc.tensor.matmul(po[:],
                                         xs[di][:, mi * 128:(mi + 1) * 128],
                                         E[di][:], start=(di == 0),
                                         stop=(di == PCC - 1))
                    nc.scalar.copy(num[h * DC + mi][:], po[:])
            # AllReduce numerator+denominator
            bin_ = drm.tile([D2 + H, ND], F32, name=f"{name}ari")
            bout = drm.tile([D2 + H, ND], F32, name=f"{name}aro", addr_space="Shared")
            for t in range(D2C):
                nc.sync.dma_start(bin_[t * 128:(t + 1) * 128, :], num[t][:])
            for h in range(H):
                nc.sync.dma_start(bin_[D2 + h:D2 + h + 1, :], den[h][:])
            nc.gpsimd.collective_compute(
                "AllReduce", mybir.AluOpType.add,
                replica_groups=[list(range(NCORE))],
                ins=[bin_[:].opt()], outs=[bout[:].opt()])
            numr = [P.tile([128, ND], F32, name=f"{name}nr{t}", tag="sh_nm",
                           bufs=D2C + 4) for t in range(D2C)]
            for t in range(D2C):
                nc.sync.dma_start(numr[t][:], bout[t * 128:(t + 1) * 128, :])
            rcps = []
            for h in range(H):
                denr = P.tile([1, ND], F32, name=f"{name}denr{h}", tag=f"sh_dr{h}", bufs=1)
                nc.sync.dma_start(denr[:], bout[D2 + h:D2 + h + 1, :])
                den2 = P.tile([1, ND], F32, name=f"{name}den2{h}", tag=f"sh_d2{h}", bufs=1)
                nc.scalar.activation(den2[:], denr[:], Act.Identity,
                                     bias=eps1[:])
                rcp = P.tile([1, ND], F32, name=f"{name}rcp{h}", tag=f"sh_rc{h}", bufs=1)
                nc.vector.reciprocal(rcp[:], den2[:])
                rcps.append(rcp)
            gat = [P.tile([128, ND], F32R, name=f"{name}g{t}", tag="sh_g",
                          bufs=D2C) for t in range(D2C)]
            for h in range(H):
                rb = bcast_sb(rcps[h], 0, ND, P, "sh_rb")
                for mi in range(DC):
                    tmp = P.tile([128, ND], F32, name=f"{name}t", tag="sh_t",
                                 bufs=2)
                    nc.vector.tensor_mul(tmp[:], numr[h * DC + mi][:], rb[:])
                    nc.scalar.activation(gat[h * DC + mi][:], tmp[:], Act.Relu,
                                         bias=gb[gi][:, h * DC + mi, :])
            dump(f"{name}_gat", gat, ND)
            r1 = [P.tile([128, ND], F32R, name=f"{name}r1{i}", tag="sh_r1",
                          bufs=DC) for i in range(DC)]
            ft_gemm(d[f"g{gi}_ltW"], D2, D,
                    lambda ki, n0, nn: gat[ki][:, n0:n0 + nn], ND,
                    ev_bias(r1, Act.Relu, ltb[gi]))
            x2 = [P.tile([128, ND], F32R, name=f"{name}x2{i}", tag="sh_x2",
                          bufs=DC) for i in range(DC)]
            ft_gemm(d[f"g{gi}_ffW"], D, D,
                    lambda ki, n0, nn: r1[ki][:, n0:n0 + nn], ND,
                    ev_bias(x2, Act.Identity, ffb[gi]))
            ft_gemm(d[f"g{gi}_pcW"], D2, D,
                    lambda ki, n0, nn: (dstT[ki] if ki < DC
                                        else x2[ki - DC])[:, n0:n0 + nn], ND,
                    ev_bias(outT, Act.Relu, pcb[gi]))

    with tc.tile_pool(name="hprep", bufs=1) as hq:
        with tc.tile_pool(name="shrd", bufs=1) as SH:
            gat_sharded(SH, 1, u0T, d["LC2"], NU, u1T, "v2c")
            for mi in range(DC):
                nc.sync.dma_start(d["u1T"][mi * 128:(mi + 1) * 128, :],
                                  u1T[mi][:].bitcast(F32))
            # head prep that only needs u1 - fills the p2p AllReduce gap
            fc1S = stage_km(hq, d["fc1W"], D, 384, "fc1S")
            fc2S = stage_km(hq, d["fc2W"], 384, 128, "fc2S")
            SselS = [hq.tile([128, UC], F32R, name=f"ssel{s}") for s in range(UCC)]
            for s in range(UCC):
                nc.sync.dma_start(SselS[s][:],
                                  d["Ssel"][s * 128:(s + 1) * 128, :].bitcast(F32R))
            ubn = [hq.tile([128, D], F32R, name=f"ubn{s}") for s in range(UCC)]
            for (n0, nn) in _nt(D):
                pos = [pmm([128, nn]) for _ in range(UCC)]
                for ki in range(DC):
                    w = wk.tile([128, 512], F32R, name="wchunk2", tag="wchunk2",
                                bufs=6)
                    nc.sync.dma_start(
                        w[:, 0:nn],
                        d["fc0b"][ki * 128:(ki + 1) * 128, n0:n0 + nn].bitcast(F32R))
                    for s in range(UCC):
                        nc.tensor.matmul(pos[s][:], u1T[ki][:, s * 128:(s + 1) * 128],
                                         w[:, 0:nn], start=(ki == 0),
                                         stop=(ki == DC - 1))
                for s in range(UCC):
                    nc.scalar.copy(ubn[s][:, n0:n0 + nn], pos[s][:])
            ubsl = [hq.tile([128, UC], F32, name=f"ubsl{mi}") for mi in range(DC)]
            for mi in range(DC):
                po = pmm([128, UC])
                for s in range(UCC):
                    nc.tensor.matmul(po[:], ubn[s][:, mi * 128:(mi + 1) * 128],
                                     SselS[s][:], start=(s == 0), stop=(s == UCC - 1))
                nc.scalar.copy(ubsl[mi][:], po[:])
            gat_sharded(SH, 2, pe0T, d["LC3"], NK, pe1T, "p2p")
            dump("u1T", u1T, NU)
            dump("pe1T", pe1T, NK)
            dump("ubsl", ubsl, UC)

        # ============ stage 5: head MLP (reuses the shrd address space) =====
        with tc.tile_pool(name="head", bufs=1) as hp:
            peaT = [hp.tile([128, NK], F32, name=f"peaT{i}") for i in range(DC)]
            ft_gemm(d["fc0a"], D, D, lambda ki, n0, nn: pe1T[ki][:, n0:n0 + nn], NK,
                    ev_bias(peaT, Act.Identity, b0S))
            fc3S = hp.tile([128, 1], F32R, name="fc3S")
            nc.sync.dma_start(fc3S[:], d["fc3W"].bitcast(F32R))
            dump("peaT", peaT, NK)
            for grp in range(NPAIR // 512):
                h0 = hp.tile([128, DC, 512], F32R, name="h0g", tag="h0g", bufs=2)
                for ci in range(DC):
                    for il in range(4):
                        nc.vector.tensor_scalar(
                            h0[:, ci, il * 128:(il + 1) * 128], peaT[ci][:],
                            ubsl[ci][:, grp * 4 + il:grp * 4 + il + 1], 0.0,
                            AluOpType.add, AluOpType.max)
                h1 = hp.tile([128, 3, 512], F32R, name="h1g", tag="h1g", bufs=2)
                for mi in range(3):
                    po = pmm([128, 512])
                    for ki in range(DC):
                        nc.tensor.matmul(po[:], fc1S[ki][:, mi * 128:(mi + 1) * 128],
                                         h0[:, ki, :], start=(ki == 0),
                                         stop=(ki == DC - 1))
                    nc.scalar.activation(h1[:, mi, :], po[:], Act.Relu,
                                         bias=fc1bS[:, mi, :])
                h2 = hp.tile([128, 512], F32R, name="h2g", tag="h2g", bufs=2)
                po = pmm([128, 512])
                for ki in range(3):
                    nc.tensor.matmul(po[:], fc2S[ki][:], h1[:, ki, :],
                                     start=(ki == 0), stop=(ki == 2))
                nc.scalar.activation(h2[:], po[:], Act.Relu, bias=fc2bS[:, 0, :])
                pd = pdn([1, 512])
                nc.tensor.matmul(pd[:], fc3S[:], h2[:], start=True, stop=True)
                yt = hp.tile([1, 512], F32, name="yt", tag="yt", bufs=2)
                nc.scalar.activation(yt[:], pd[:], Act.Sigmoid, bias=fc3bS[0:1, :])
                nc.sync.dma_start(d["y"][:, grp * 512:(grp + 1) * 512], yt[:])
    ctx.close()


# ================= host side =================

def _prep(inputs):
    """Host-side preprocessing: padding, transposes, BN folding, edge->mask."""
    def deep(v):
        if isinstance(v, dict):
            return {k: deep(x) for k, x in v.items()}
        return np.asarray(v)

    inputs = deep(inputs)
    p = inputs["params"]
    f32 = np.float32

    def padK(W):  # [F_IN, M] -> [FP, M]
        W = np.asarray(W, f32)
        return np.concatenate([W, np.zeros((FP - F_IN, W.shape[1]), f32)], 0)

    def padKT(X):  # [N, F_IN] -> [FP, N]
        X = np.asarray(X, f32).T
        return np.concatenate([X, np.zeros((FP - F_IN, X.shape[1]), f32)], 0)

    rep = {"c_ones": np.ones((128, 128), f32)}
    rep["ufT"] = padKT(inputs["user_features"])
    rep["kfT"] = padKT(inputs["persona_features"])
    rep["W_user"] = padK(p["W_user"])
    rep["W_prod"] = padK(p["W_prod"])
    rep["W_pers"] = padK(p["W_pers"])
    rep["b_user"] = np.asarray(p["b_user"], f32).reshape(D, 1)
    rep["b_prod"] = np.asarray(p["b_prod"], f32).reshape(D, 1)
    rep["b_pers"] = np.asarray(p["b_pers"], f32).reshape(D, 1)
    for g, key in enumerate(("c2v", "v2c", "p2p")):
        gp = p[key]
        Ws = np.asarray(gp["Ws"], f32)
        Wd = np.asarray(gp["Wd"], f32)
        att_s = np.asarray(gp["att_s"], f32)
        att_d = np.asarray(gp["att_d"], f32)
        rep[f"g{g}_Ws"] = Ws
        rep[f"g{g}_Wsatt"] = np.stack(
            [Ws[:, h * D:(h + 1) * D] @ att_s[h] for h in range(H)], 1)
        rep[f"g{g}_Wdatt"] = np.stack(
            [Wd[:, h * D:(h + 1) * D] @ att_d[h] for h in range(H)], 1)
        rep[f"g{g}_gbias"] = np.asarray(gp["bias"], f32).reshape(D2, 1)
        rep[f"g{g}_ltW"] = np.asarray(gp["lt_W"], f32)
        rep[f"g{g}_ltb"] = np.asarray(gp["lt_b"], f32).reshape(D, 1)
        rep[f"g{g}_ffW"] = np.asarray(gp["ff_W"], f32)
        rep[f"g{g}_ffb"] = np.asarray(gp["ff_b"], f32).reshape(D, 1)
        rep[f"g{g}_pcW"] = np.asarray(gp["pc_W"], f32)
        rep[f"g{g}_pcb"] = np.asarray(gp["pc_b"], f32).reshape(D, 1)
    s0 = f32(1.0 / np.sqrt(1.0 + 1e-5))
    g0 = np.asarray(p["bn0_g"], f32) * s0
    g1 = np.asarray(p["bn1_g"], f32) * s0
    g2 = np.asarray(p["bn2_g"], f32) * s0
    fc0 = np.asarray(p["fc0_W"], f32)
    rep["fc0a"] = fc0[:D] * g0[None, :]
    rep["fc0b"] = fc0[D:] * g0[None, :]
    rep["b0"] = (np.asarray(p["fc0_b"], f32) * g0
                 + np.asarray(p["bn0_b"], f32)).reshape(D, 1)
    rep["fc1W"] = np.asarray(p["fc1_W"], f32) * g1[None, :]
    rep["fc1b"] = (np.asarray(p["fc1_b"], f32) * g1
                   + np.asarray(p["bn1_b"], f32)).reshape(384, 1)
    rep["fc2W"] = np.asarray(p["fc2_W"], f32) * g2[None, :]
    rep["fc2b"] = (np.asarray(p["fc2_b"], f32) * g2
                   + np.asarray(p["bn2_b"], f32)).reshape(128, 1)
    rep["fc3W"] = np.asarray(p["fc3_W"], f32).reshape(128, 1)
    rep["fc3b"] = np.asarray(p["fc3_b"], f32).reshape(1, 1)

    # edge lists -> dense log-count masks
    ei = np.asarray(inputs["edge_indices"]).astype(np.int64)
    ei0 = ei[0] - ei[0].min()       # product (dst of c2v)
    ei1 = ei[1]                     # user
    C1 = np.zeros((NP, NU), f32)
    np.add.at(C1, (ei0, ei1), f32(1.0))
    pp = np.asarray(inputs["persona_prod_edge_ind"]).astype(np.int64)
    pp0 = pp[0]                     # product (src of p2p)
    pp1 = pp[1] - pp[1].min()       # persona (dst)
    C2 = np.zeros((NK, NP), f32)
    np.add.at(C2, (pp1, pp0), f32(1.0))

    def logmask(C):
        out = np.full(C.shape, MASK, f32)
        nz = C > 0
        out[nz] = np.log(C[nz])
        return out

    LC1 = logmask(C1)               # [NP, NU]
    LC3 = logmask(C2.T)             # [NP, NK]
    pfT = padKT(inputs["product_features"])

    in_maps = []
    for c in range(NCORE):
        sl = slice(c * PC, (c + 1) * PC)
        m = dict(rep)
        m["pfT"] = np.ascontiguousarray(pfT[:, sl])
        m["LC1T"] = np.ascontiguousarray(LC1[sl].T)
        m["LC2"] = np.ascontiguousarray(LC1[sl])
        m["LC3"] = np.ascontiguousarray(LC3[sl])
        S = np.zeros((NU, UC), f32)
        S[np.arange(c * UC, (c + 1) * UC), np.arange(UC)] = 1.0
        m["Ssel"] = S
        in_maps.append(m)
    return in_maps


_CACHE = {}
TRACE = False
DEBUG = False
LAST = {}


def kernel(**inputs):
    key = ("nc", DEBUG)
    if key not in _CACHE:
        _CACHE[key] = _build(debug=DEBUG)
    nc = _CACHE[key]
    in_maps = _prep(inputs)
    res = run_bass_kernel_spmd(nc, in_maps, core_ids=list(range(NCORE)),
                               trace=TRACE)
    LAST["res"] = res
    r = res.results
    x = np.concatenate([r[c]["y"][0] for c in range(NCORE)]).reshape(NU * NK, 1)
    u = np.ascontiguousarray(r[0]["u1T"].T)
    pr = np.concatenate([r[c]["pr1T"].T for c in range(NCORE)], 0)
    return x.astype(np.float32), u.astype(np.float32), pr.astype(np.float32)
